# revision 18
# baseline (speedup 1.0000x reference)
"""Binarized MLP (64->2048->1024->512->64->1, B=32768) on 8 trn2 NeuronCores.

Strategy (data-parallel over batch, weights replicated):
- Activations after each binarized layer are exactly {0,1}; binarized weights
  are exactly {-1,0,+1}. Layers 2-4 run in fp8 DoubleRow with exact fp32 PSUM
  accumulation.
- x is transposed and 2-term bf16 split (residual ~|x|*2^-18) on the host,
  shipped as one [128, bc] bf16 tensor with both terms stacked on the
  partition dim, so each 128-feature output tile needs ONE K=128 bf16 matmul
  and the device does no transposes or split arithmetic at all. (fp8
  DoubleRow for L1 was tried and reverted: the DR adder tree accumulates at
  ~13-bit mantissa, exact for the integer-valued sums of L2-4 but too lossy
  for real-valued L1.) Layer 5 uses a single bf16 copy of w5 (error ~2^-9
  relative, washed out by the sigmoid under the rel-err budget).
- The next tile's L1 matmuls are interleaved between the current tile's L2
  m-tiles so L1's epilogues drain while the PE is busy with DoubleRow work
  instead of backpressuring it through the PSUM pool.
- BN(eval) + bias + hardtanh + 1-bit actq collapse into a per-feature
  threshold: out_bit = (matmul > thr), thr = m - be*sqrt(v+eps)/g - b.
- Activations are kept feature-major on chip ([feat, batch]).
- Threshold epilogues alternate DVE (is_gt -> {0,1}) and ACT (Sign -> {-1,+1})
  per 128-feature tile. ACT-coded features get next-layer weight columns
  scaled by 1/2 (exact in fp8) plus a host-side threshold correction.
- Per-tile sigmoid writes column slices of one [1, 4096] output tile so the
  rep loop issues a single output DMA.
"""

import sys

import numpy as np

sys.path.insert(0, "/opt/trn_rl_repo")

import ml_dtypes

import concourse.bacc as bacc
import concourse.mybir as mybir
import concourse.tile as tile
from concourse.bass import ts, ds
from concourse.bass_utils import run_bass_kernel_spmd
from contextlib import ExitStack

N_CORES = 8
B = 32768
BC = B // N_CORES          # 4096 rows per core
BT = 512                   # batch tile (free dim of matmuls)
EPS = 1e-5

F32 = mybir.dt.float32
BF16 = mybir.dt.bfloat16
FP8 = mybir.dt.float8e4
DR = mybir.MatmulPerfMode.DoubleRow


def _thr(b, g, be, m, v):
    # (z + b - m) * g/sqrt(v+eps) + be > 0  <=>  z > m - be*sqrt(v+eps)/g - b
    s = np.float64(g) / np.sqrt(np.float64(v) + EPS)
    return np.float64(m) - np.float64(be) / s - np.float64(b)


def _feat_major(a, n_feat):
    # [n_feat(, rest)] -> [128, n_feat//128(, rest)] with feature f at
    # [f % 128, f // 128]
    ks = n_feat // 128
    return np.ascontiguousarray(a.reshape((ks, 128) + a.shape[1:]).swapaxes(0, 1))


def prep_x(xc):
    """[bc, 64] f32 -> [128, bc] bf16: x^T 2-term bf16 split stacked on the
    partition dim (rows 0-63 high term, 64-127 residual term)."""
    bf = ml_dtypes.bfloat16
    xt = np.ascontiguousarray(xc.astype(np.float32).T)      # [64, bc]
    hi = xt.astype(bf)
    lo = (xt - hi.astype(np.float32)).astype(bf)
    return np.ascontiguousarray(np.concatenate([hi, lo], axis=0))


def _is_act_tile(kt):
    # m-tile kt of a layer's output features: DVE ({0,1}) if even, ACT ({-1,1})
    return kt % 2 == 1


def build_program(bc=BC, bt=BT, reps=1, dummies=True, interleave=True, psmm=7):
    nbt = bc // bt
    nc = bacc.Bacc("TRN2", target_bir_lowering=False)

    x_d = nc.declare_dram_parameter("x", [128, bc], BF16, False)
    w1t_d = nc.declare_dram_parameter("w1t", [128, 2048], BF16, False)
    w2t_d = nc.declare_dram_parameter("w2t", [128, 16, 1024], FP8, False)
    w3t_d = nc.declare_dram_parameter("w3t", [128, 8, 512], FP8, False)
    w4t_d = nc.declare_dram_parameter("w4t", [128, 4, 64], FP8, False)
    w5t_d = nc.declare_dram_parameter("w5t", [64, 1], BF16, False)
    thr1_d = nc.declare_dram_parameter("thr1", [128, 16], F32, False)
    thr2_d = nc.declare_dram_parameter("thr2", [128, 8], F32, False)
    thr3_d = nc.declare_dram_parameter("thr3", [128, 4], F32, False)
    thr4_d = nc.declare_dram_parameter("thr4", [64, 1], F32, False)
    nthr1_d = nc.declare_dram_parameter("nthr1", [128, 16], F32, False)
    nthr2_d = nc.declare_dram_parameter("nthr2", [128, 8], F32, False)
    nthr3_d = nc.declare_dram_parameter("nthr3", [128, 4], F32, False)
    b5_d = nc.declare_dram_parameter("b5", [1, 1], F32, False)
    out_d = nc.declare_dram_parameter("out", [nbt, bt], F32, True)

    gt = mybir.AluOpType.is_gt
    SIGN = mybir.ActivationFunctionType.Sign
    SIGMOID = mybir.ActivationFunctionType.Sigmoid

    with tile.TileContext(nc) as tc:
        with ExitStack() as ctx:
            const = ctx.enter_context(tc.tile_pool(name="const", bufs=1))
            xb_p = ctx.enter_context(tc.tile_pool(name="xb", bufs=3))
            h1_p = ctx.enter_context(tc.tile_pool(name="h1", bufs=3))
            h2_p = ctx.enter_context(tc.tile_pool(name="h2", bufs=3))
            h3_p = ctx.enter_context(tc.tile_pool(name="h3", bufs=2))
            h4_p = ctx.enter_context(tc.tile_pool(name="h4", bufs=2))
            o_p = ctx.enter_context(tc.tile_pool(name="o", bufs=2))
            ps_mm = ctx.enter_context(tc.tile_pool(name="psmm", bufs=psmm, space="PSUM"))
            ps_tp = ctx.enter_context(tc.tile_pool(name="pstp", bufs=8 - psmm, space="PSUM"))

            def cload(nm, shape, dtype, dram):
                t = const.tile(shape, dtype, tag=nm, name=nm)
                nc.sync.dma_start(t[:], dram[:])
                return t

            w1t = cload("w1t", [128, 2048], BF16, w1t_d)
            w2t = cload("w2t", [128, 16, 1024], FP8, w2t_d)
            w3t = cload("w3t", [128, 8, 512], FP8, w3t_d)
            w4t = cload("w4t", [128, 4, 64], FP8, w4t_d)
            w5t = cload("w5t", [64, 1], BF16, w5t_d)
            thr1 = cload("thr1", [128, 16], F32, thr1_d)
            thr2 = cload("thr2", [128, 8], F32, thr2_d)
            thr3 = cload("thr3", [128, 4], F32, thr3_d)
            thr4 = cload("thr4", [64, 1], F32, thr4_d)
            nthr1 = cload("nthr1", [128, 16], F32, nthr1_d)
            nthr2 = cload("nthr2", [128, 8], F32, nthr2_d)
            nthr3 = cload("nthr3", [128, 4], F32, nthr3_d)
            b5 = cload("b5", [1, 1], F32, b5_d)

            # --- dummy consumers: absorb every const-producing semaphore so
            # steady-state matmuls/epilogues carry at most one wait each ---
            if dummies:
                dps = ps_tp.tile([128, 128], F32, tag="tp")
                nc.tensor.matmul(dps[:], lhsT=w1t[:, 0:128], rhs=w1t[:, 0:128],
                                 start=True, stop=True)
                nc.tensor.matmul(dps[:], lhsT=w2t[:, 0, 0:128], rhs=w2t[:, 0, 0:128],
                                 start=True, stop=True)
                nc.tensor.matmul(dps[:], lhsT=w3t[:, 0, 0:128], rhs=w3t[:, 0, 0:128],
                                 start=True, stop=True)
                nc.tensor.matmul(dps[:64, :64], lhsT=w4t[:, 0, :], rhs=w4t[:, 0, :],
                                 start=True, stop=True)
                nc.tensor.matmul(dps[:1, :1], lhsT=w5t[:], rhs=w5t[:],
                                 start=True, stop=True)
                dsb = const.tile([128, 16], F32)
                nc.vector.tensor_copy(dsb[:, 0:16], thr1[:])
                nc.vector.tensor_copy(dsb[:, 0:8], thr2[:])
                nc.vector.tensor_copy(dsb[:, 0:4], thr3[:])
                nc.vector.tensor_copy(dsb[:64, 0:1], thr4[:])
                dsb2 = const.tile([128, 16], F32)
                nc.scalar.copy(dsb2[:, 0:16], nthr1[:])
                nc.scalar.copy(dsb2[:, 0:8], nthr2[:])
                nc.scalar.copy(dsb2[:, 0:4], nthr3[:])
                nc.scalar.copy(dsb2[:1, 0:1], b5[:])

            def mm_layer(ps, w, h, mt, nk):
                msl = slice(None) if mt is None else ts(mt, 128)
                for k in range(0, nk, 2):
                    nc.tensor.matmul(ps[:], lhsT=w[:, k : k + 2, msl],
                                     rhs=h[:, k : k + 2, :], perf_mode=DR,
                                     start=(k == 0), stop=(k == nk - 2))

            def epilogue(h_ap, ps, mt, thr, nthr):
                if _is_act_tile(mt):
                    nc.scalar.activation(h_ap, ps[:], SIGN,
                                         bias=nthr[:, mt : mt + 1], scale=1.0)
                else:
                    nc.vector.tensor_scalar(h_ap, ps[:], thr[:, mt : mt + 1],
                                            None, gt)

            def front_a(b):
                """load the host-split x tile: rows 0-63 = bf16(x^T),
                rows 64-127 = bf16 residual (2-term split, done host-side)"""
                xb = xb_p.tile([128, bt], BF16, tag="xb", name="xb")
                nc.sync.dma_start(xb[:], x_d[:, ds(b * bt, bt)])
                return xb

            def l1_mts(h1, xb, mts):
                """L1 matmuls + thresholds for the given m-tiles -> h1"""
                for mt in mts:
                    ps = ps_mm.tile([128, bt], F32, tag="mm", name="ps")
                    nc.tensor.matmul(ps[:], lhsT=w1t[:, ts(mt, 128)], rhs=xb[:],
                                     start=True, stop=True)
                    epilogue(h1[:, mt, :], ps, mt, thr1, nthr1)

            def stage_back(b, h1, h1_next, xb_next, o):
                """L2..L5 + sigmoid into o[:, b*bt:(b+1)*bt] for batch tile b.

                The next tile's L1 matmuls are interleaved between L2 m-tiles:
                during L2 the epilogue engines are underloaded, so the L1
                epilogues drain there instead of backpressuring the PE."""
                h2 = h2_p.tile([128, 8, bt], FP8, tag="h2", name="h2")
                for mt in range(8):
                    ps = ps_mm.tile([128, bt], F32, tag="mm", name="ps")
                    mm_layer(ps, w2t, h1, mt, 16)
                    epilogue(h2[:, mt, :], ps, mt, thr2, nthr2)
                    if interleave and xb_next is not None:
                        l1_mts(h1_next, xb_next, [2 * mt, 2 * mt + 1])

                h3 = h3_p.tile([128, 4, bt], FP8, tag="h3", name="h3")
                for mt in range(4):
                    ps = ps_mm.tile([128, bt], F32, tag="mm", name="ps")
                    mm_layer(ps, w3t, h2, mt, 8)
                    epilogue(h3[:, mt, :], ps, mt, thr3, nthr3)

                h4 = h4_p.tile([64, bt], BF16, tag="h4", name="h4")
                ps4 = ps_mm.tile([64, bt], F32, tag="mm", name="ps4")
                mm_layer(ps4, w4t, h3, None, 4)
                nc.vector.tensor_scalar(h4[:], ps4[:], thr4[:, 0:1], None, gt)

                ps5 = ps_mm.tile([1, bt], F32, tag="mm", name="ps5")
                nc.tensor.matmul(ps5[:], lhsT=w5t[:], rhs=h4[:],
                                 start=True, stop=True)
                nc.scalar.activation(o[:, ts(b, bt)], ps5[:], SIGMOID,
                                     bias=b5[:1, :1], scale=1.0)

            rep_ctx = tc.For_i(0, reps, 1) if reps > 1 else None
            if rep_ctx is not None:
                rep_ctx.__enter__()

            o = o_p.tile([1, nbt * bt], F32, tag="o", name="o")
            h1_cur = h1_p.tile([128, 16, bt], FP8, tag="h1", name="h1")
            l1_mts(h1_cur, front_a(0), range(16))
            for b in range(nbt):
                if b + 1 < nbt:
                    xb_next = front_a(b + 1)
                    h1_next = h1_p.tile([128, 16, bt], FP8, tag="h1", name="h1")
                else:
                    xb_next = h1_next = None
                stage_back(b, h1_cur, h1_next, xb_next, o)
                if not interleave and xb_next is not None:
                    l1_mts(h1_next, xb_next, range(16))
                h1_cur = h1_next
            nc.sync.dma_start(out_d[:], o[:])

            if rep_ctx is not None:
                rep_ctx.__exit__(None, None, None)

    nc.compile()
    return nc


def prep_weights(w1, b1, w2, b2, w3, b3, w4, b4, w5, b5,
                 *, g1, be1, m1, v1, g2, be2, m2, v2,
                 g3, be3, m3, v3, g4, be4, m4, v4):
    bf = ml_dtypes.bfloat16
    f8 = ml_dtypes.float8_e4m3
    f64 = np.float64

    # layer 1: sign weights duplicated on both K-halves (for the stacked
    # 2-term bf16 split of x); no input coding.
    w1b = np.sign(w1).astype(f64)                                       # [2048,64]
    thr1 = _thr(b1, g1, be1, m1, v1)                                    # [2048]

    def scaled(wb, thr_next, n_in):
        """Scale ACT-coded input columns by 1/2 and fold the matching
        -0.5*sum(sign) correction into the next layer's threshold."""
        wb = wb.copy()
        corr = np.zeros(wb.shape[0], f64)
        for kt in range(n_in // 128):
            if _is_act_tile(kt):
                cols = slice(kt * 128, (kt + 1) * 128)
                corr += wb[:, cols].sum(axis=1) * 0.5
                wb[:, cols] *= 0.5
        return wb, thr_next - corr

    w2b, thr2 = scaled(np.sign(w2).astype(f64), _thr(b2, g2, be2, m2, v2), 2048)
    w3b, thr3 = scaled(np.sign(w3).astype(f64), _thr(b3, g3, be3, m3, v3), 1024)
    w4b, thr4 = scaled(np.sign(w4).astype(f64), _thr(b4, g4, be4, m4, v4), 512)

    out = dict(
        w1t=np.ascontiguousarray(np.concatenate([w1b.T, w1b.T], axis=0).astype(bf)),  # [128,2048]
        w2t=_feat_major(w2b.T.astype(f8), 2048),                        # [128,16,1024]
        w3t=_feat_major(w3b.T.astype(f8), 1024),                        # [128,8,512]
        w4t=_feat_major(w4b.T.astype(f8), 512),                         # [128,4,64]
        w5t=np.ascontiguousarray(np.asarray(w5, np.float32).reshape(64, 1).astype(bf)),  # [64,1]
        thr1=_feat_major(thr1.astype(np.float32), 2048),
        thr2=_feat_major(thr2.astype(np.float32), 1024),
        thr3=_feat_major(thr3.astype(np.float32), 512),
        thr4=np.ascontiguousarray(thr4.astype(np.float32).reshape(64, 1)),
        nthr1=_feat_major((-thr1).astype(np.float32), 2048),
        nthr2=_feat_major((-thr2).astype(np.float32), 1024),
        nthr3=_feat_major((-thr3).astype(np.float32), 512),
        b5=np.asarray(b5, np.float32).reshape(1, 1),
    )
    return out


_CACHED = {}


def run(inputs, trace=False):
    if "nc" not in _CACHED:
        _CACHED["nc"] = build_program()
    nc = _CACHED["nc"]

    x = np.asarray(inputs["x"], np.float32)
    wmap = prep_weights(**{k: np.asarray(v) for k, v in inputs.items() if k != "x"})
    in_maps = []
    for c in range(N_CORES):
        m = dict(wmap)
        m["x"] = prep_x(x[c * BC : (c + 1) * BC])
        in_maps.append(m)

    res = run_bass_kernel_spmd(nc, in_maps, list(range(N_CORES)), trace=trace)
    out = np.concatenate(
        [np.asarray(r["out"]).reshape(BC, 1) for r in res.results], axis=0
    )
    return out, res


def kernel(**inputs) -> np.ndarray:
    out, _ = run(inputs, trace=False)
    return out


# revision 19
# speedup vs baseline: 1.2190x; 1.2190x over previous
"""Binarized MLP (64->2048->1024->512->64->1, B=32768) on 8 trn2 NeuronCores.

Strategy (data-parallel over batch, weights replicated):
- Activations after each binarized layer are exactly {0,1}; binarized weights
  are exactly {-1,0,+1}. Layers 2-4 run in fp8 DoubleRow with exact fp32 PSUM
  accumulation.
- x is transposed and 2-term bf16 split (residual ~|x|*2^-18) on the host,
  shipped as one [128, bc] bf16 tensor with both terms stacked on the
  partition dim, so each 128-feature output tile needs ONE K=128 bf16 matmul
  and the device does no transposes or split arithmetic at all. (fp8
  DoubleRow for L1 was tried and reverted: the DR adder tree accumulates at
  ~13-bit mantissa, exact for the integer-valued sums of L2-4 but too lossy
  for real-valued L1.) Layer 5 uses a single bf16 copy of w5 (error ~2^-9
  relative, washed out by the sigmoid under the rel-err budget).
- The next tile's L1 matmuls are interleaved between the current tile's L2
  m-tiles so L1's epilogues drain while the PE is busy with DoubleRow work
  instead of backpressuring it through the PSUM pool.
- BN(eval) + bias + hardtanh + 1-bit actq collapse into a per-feature
  threshold: out_bit = (matmul > thr), thr = m - be*sqrt(v+eps)/g - b.
- Activations are kept feature-major on chip ([feat, batch]).
- Threshold epilogues alternate DVE (is_gt -> {0,1}) and ACT (Sign -> {-1,+1})
  per 128-feature tile. ACT-coded features get next-layer weight columns
  scaled by 1/2 (exact in fp8) plus a host-side threshold correction.
- Per-tile sigmoid writes column slices of one [1, 4096] output tile so the
  rep loop issues a single output DMA.
"""

import sys

import numpy as np

sys.path.insert(0, "/opt/trn_rl_repo")

import ml_dtypes

import concourse.bacc as bacc
import concourse.mybir as mybir
import concourse.tile as tile
from concourse.bass import ts, ds
from concourse.bass_utils import run_bass_kernel_spmd
from contextlib import ExitStack

N_CORES = 8
B = 32768
BC = B // N_CORES          # 4096 rows per core
BT = 512                   # batch tile (free dim of matmuls)
EPS = 1e-5

F32 = mybir.dt.float32
BF16 = mybir.dt.bfloat16
FP8 = mybir.dt.float8e4
DR = mybir.MatmulPerfMode.DoubleRow


def _thr(b, g, be, m, v):
    # (z + b - m) * g/sqrt(v+eps) + be > 0  <=>  z > m - be*sqrt(v+eps)/g - b
    s = np.float64(g) / np.sqrt(np.float64(v) + EPS)
    return np.float64(m) - np.float64(be) / s - np.float64(b)


def _feat_major(a, n_feat):
    # [n_feat(, rest)] -> [128, n_feat//128(, rest)] with feature f at
    # [f % 128, f // 128]
    ks = n_feat // 128
    return np.ascontiguousarray(a.reshape((ks, 128) + a.shape[1:]).swapaxes(0, 1))


def prep_x(xc):
    """[bc, 64] f32 -> [128, bc] bf16: x^T 2-term bf16 split stacked on the
    partition dim (rows 0-63 high term, 64-127 residual term)."""
    bf = ml_dtypes.bfloat16
    xt = np.ascontiguousarray(xc.astype(np.float32).T)      # [64, bc]
    hi = xt.astype(bf)
    lo = (xt - hi.astype(np.float32)).astype(bf)
    return np.ascontiguousarray(np.concatenate([hi, lo], axis=0))


def _is_act_tile(kt):
    # m-tile kt of a layer's output features: DVE ({0,1}) if even, ACT ({-1,1})
    return kt % 2 == 1


def build_program(bc=BC, bt=BT, reps=1, dummies=True, interleave=True, psmm=7):
    nbt = bc // bt
    nc = bacc.Bacc("TRN2", target_bir_lowering=False)

    x_d = nc.declare_dram_parameter("x", [128, bc], BF16, False)
    w1t_d = nc.declare_dram_parameter("w1t", [128, 2048], BF16, False)
    w2t_d = nc.declare_dram_parameter("w2t", [128, 16, 1024], FP8, False)
    w3t_d = nc.declare_dram_parameter("w3t", [128, 8, 512], FP8, False)
    w4t_d = nc.declare_dram_parameter("w4t", [128, 4, 64], FP8, False)
    w5t_d = nc.declare_dram_parameter("w5t", [64, 1], BF16, False)
    thr1_d = nc.declare_dram_parameter("thr1", [128, 16], F32, False)
    thr2_d = nc.declare_dram_parameter("thr2", [128, 8], F32, False)
    thr3_d = nc.declare_dram_parameter("thr3", [128, 4], F32, False)
    thr4_d = nc.declare_dram_parameter("thr4", [64, 1], F32, False)
    nthr1_d = nc.declare_dram_parameter("nthr1", [128, 16], F32, False)
    nthr2_d = nc.declare_dram_parameter("nthr2", [128, 8], F32, False)
    nthr3_d = nc.declare_dram_parameter("nthr3", [128, 4], F32, False)
    b5_d = nc.declare_dram_parameter("b5", [1, 1], F32, False)
    out_d = nc.declare_dram_parameter("out", [nbt, bt], F32, True)

    gt = mybir.AluOpType.is_gt
    SIGN = mybir.ActivationFunctionType.Sign
    SIGMOID = mybir.ActivationFunctionType.Sigmoid

    with tile.TileContext(nc) as tc:
        with ExitStack() as ctx:
            const = ctx.enter_context(tc.tile_pool(name="const", bufs=1))
            xb_p = ctx.enter_context(tc.tile_pool(name="xb", bufs=3))
            h1_p = ctx.enter_context(tc.tile_pool(name="h1", bufs=3))
            h2_p = ctx.enter_context(tc.tile_pool(name="h2", bufs=3))
            h3_p = ctx.enter_context(tc.tile_pool(name="h3", bufs=2))
            h4_p = ctx.enter_context(tc.tile_pool(name="h4", bufs=2))
            o_p = ctx.enter_context(tc.tile_pool(name="o", bufs=2))
            ps_mm = ctx.enter_context(tc.tile_pool(name="psmm", bufs=psmm, space="PSUM"))
            ps_tp = ctx.enter_context(tc.tile_pool(name="pstp", bufs=8 - psmm, space="PSUM"))

            def cload(nm, shape, dtype, dram):
                t = const.tile(shape, dtype, tag=nm, name=nm)
                nc.sync.dma_start(t[:], dram[:])
                return t

            w1t = cload("w1t", [128, 2048], BF16, w1t_d)
            w2t = cload("w2t", [128, 16, 1024], FP8, w2t_d)
            w3t = cload("w3t", [128, 8, 512], FP8, w3t_d)
            w4t = cload("w4t", [128, 4, 64], FP8, w4t_d)
            w5t = cload("w5t", [64, 1], BF16, w5t_d)
            thr1 = cload("thr1", [128, 16], F32, thr1_d)
            thr2 = cload("thr2", [128, 8], F32, thr2_d)
            thr3 = cload("thr3", [128, 4], F32, thr3_d)
            thr4 = cload("thr4", [64, 1], F32, thr4_d)
            nthr1 = cload("nthr1", [128, 16], F32, nthr1_d)
            nthr2 = cload("nthr2", [128, 8], F32, nthr2_d)
            nthr3 = cload("nthr3", [128, 4], F32, nthr3_d)
            b5 = cload("b5", [1, 1], F32, b5_d)

            # --- dummy consumers: absorb every const-producing semaphore so
            # steady-state matmuls/epilogues carry at most one wait each ---
            if dummies:
                dps = ps_tp.tile([128, 128], F32, tag="tp")
                nc.tensor.matmul(dps[:], lhsT=w1t[:, 0:128], rhs=w1t[:, 0:128],
                                 start=True, stop=True)
                nc.tensor.matmul(dps[:], lhsT=w2t[:, 0, 0:128], rhs=w2t[:, 0, 0:128],
                                 start=True, stop=True)
                nc.tensor.matmul(dps[:], lhsT=w3t[:, 0, 0:128], rhs=w3t[:, 0, 0:128],
                                 start=True, stop=True)
                nc.tensor.matmul(dps[:64, :64], lhsT=w4t[:, 0, :], rhs=w4t[:, 0, :],
                                 start=True, stop=True)
                nc.tensor.matmul(dps[:1, :1], lhsT=w5t[:], rhs=w5t[:],
                                 start=True, stop=True)
                dsb = const.tile([128, 16], F32)
                nc.vector.tensor_copy(dsb[:, 0:16], thr1[:])
                nc.vector.tensor_copy(dsb[:, 0:8], thr2[:])
                nc.vector.tensor_copy(dsb[:, 0:4], thr3[:])
                nc.vector.tensor_copy(dsb[:64, 0:1], thr4[:])
                dsb2 = const.tile([128, 16], F32)
                nc.scalar.copy(dsb2[:, 0:16], nthr1[:])
                nc.scalar.copy(dsb2[:, 0:8], nthr2[:])
                nc.scalar.copy(dsb2[:, 0:4], nthr3[:])
                nc.scalar.copy(dsb2[:1, 0:1], b5[:])

            def mm_layer(ps, w, h, mt, nk):
                msl = slice(None) if mt is None else ts(mt, 128)
                for k in range(0, nk, 2):
                    nc.tensor.matmul(ps[:], lhsT=w[:, k : k + 2, msl],
                                     rhs=h[:, k : k + 2, :], perf_mode=DR,
                                     start=(k == 0), stop=(k == nk - 2))

            def epilogue(h_ap, ps, mt, thr, nthr):
                if _is_act_tile(mt):
                    nc.scalar.activation(h_ap, ps[:], SIGN,
                                         bias=nthr[:, mt : mt + 1], scale=1.0)
                else:
                    nc.vector.tensor_scalar(h_ap, ps[:], thr[:, mt : mt + 1],
                                            None, gt)

            def front_a(b):
                """load the host-split x tile: rows 0-63 = bf16(x^T),
                rows 64-127 = bf16 residual (2-term split, done host-side)"""
                xb = xb_p.tile([128, bt], BF16, tag="xb", name="xb")
                nc.sync.dma_start(xb[:], x_d[:, ds(b * bt, bt)])
                return xb

            def l1_mts(h1, xb, mts):
                """L1 matmuls + thresholds for the given m-tiles -> h1"""
                for mt in mts:
                    ps = ps_mm.tile([128, bt], F32, tag="mm", name="ps")
                    nc.tensor.matmul(ps[:], lhsT=w1t[:, ts(mt, 128)], rhs=xb[:],
                                     start=True, stop=True)
                    epilogue(h1[:, mt, :], ps, mt, thr1, nthr1)

            def emit_l5(b, h4, o):
                """L5 matmul + sigmoid for tile b (deferred so the PE has L2
                work of the next tile while the L4 epilogue produces h4)."""
                ps5 = ps_mm.tile([1, bt], F32, tag="mm", name="ps5")
                nc.tensor.matmul(ps5[:], lhsT=w5t[:], rhs=h4[:],
                                 start=True, stop=True)
                nc.scalar.activation(o[:, ts(b, bt)], ps5[:], SIGMOID,
                                     bias=b5[:1, :1], scale=1.0)

            def stage_back(b, h1, h1_next, xb_next, o, prev_l5):
                """L2..L4 for batch tile b; returns (b, h4) for deferred L5.

                The next tile's L1 matmuls are interleaved between L2 m-tiles:
                during L2 the epilogue engines are underloaded, so the L1
                epilogues drain there instead of backpressuring the PE. The
                previous tile's L5+sigmoid is emitted after the first L2
                m-tile for the same reason."""
                h2 = h2_p.tile([128, 8, bt], FP8, tag="h2", name="h2")
                for mt in range(8):
                    ps = ps_mm.tile([128, bt], F32, tag="mm", name="ps")
                    mm_layer(ps, w2t, h1, mt, 16)
                    epilogue(h2[:, mt, :], ps, mt, thr2, nthr2)
                    if mt == 0 and prev_l5 is not None:
                        emit_l5(prev_l5[0], prev_l5[1], o)
                    if interleave and xb_next is not None:
                        l1_mts(h1_next, xb_next, [2 * mt, 2 * mt + 1])

                h3 = h3_p.tile([128, 4, bt], FP8, tag="h3", name="h3")
                for mt in range(4):
                    ps = ps_mm.tile([128, bt], F32, tag="mm", name="ps")
                    mm_layer(ps, w3t, h2, mt, 8)
                    epilogue(h3[:, mt, :], ps, mt, thr3, nthr3)

                h4 = h4_p.tile([64, bt], BF16, tag="h4", name="h4")
                ps4 = ps_mm.tile([64, bt], F32, tag="mm", name="ps4")
                mm_layer(ps4, w4t, h3, None, 4)
                nc.vector.tensor_scalar(h4[:], ps4[:], thr4[:, 0:1], None, gt)
                return (b, h4)

            rep_ctx = tc.For_i(0, reps, 1) if reps > 1 else None
            if rep_ctx is not None:
                rep_ctx.__enter__()

            o = o_p.tile([1, nbt * bt], F32, tag="o", name="o")
            h1_cur = h1_p.tile([128, 16, bt], FP8, tag="h1", name="h1")
            l1_mts(h1_cur, front_a(0), range(16))
            prev_l5 = None
            for b in range(nbt):
                if b + 1 < nbt:
                    xb_next = front_a(b + 1)
                    h1_next = h1_p.tile([128, 16, bt], FP8, tag="h1", name="h1")
                else:
                    xb_next = h1_next = None
                prev_l5 = stage_back(b, h1_cur, h1_next, xb_next, o, prev_l5)
                if not interleave and xb_next is not None:
                    l1_mts(h1_next, xb_next, range(16))
                h1_cur = h1_next
            emit_l5(prev_l5[0], prev_l5[1], o)
            nc.sync.dma_start(out_d[:], o[:])

            if rep_ctx is not None:
                rep_ctx.__exit__(None, None, None)

    nc.compile()
    return nc


def prep_weights(w1, b1, w2, b2, w3, b3, w4, b4, w5, b5,
                 *, g1, be1, m1, v1, g2, be2, m2, v2,
                 g3, be3, m3, v3, g4, be4, m4, v4):
    bf = ml_dtypes.bfloat16
    f8 = ml_dtypes.float8_e4m3
    f64 = np.float64

    # layer 1: sign weights duplicated on both K-halves (for the stacked
    # 2-term bf16 split of x); no input coding.
    w1b = np.sign(w1).astype(f64)                                       # [2048,64]
    thr1 = _thr(b1, g1, be1, m1, v1)                                    # [2048]

    def scaled(wb, thr_next, n_in):
        """Scale ACT-coded input columns by 1/2 and fold the matching
        -0.5*sum(sign) correction into the next layer's threshold."""
        wb = wb.copy()
        corr = np.zeros(wb.shape[0], f64)
        for kt in range(n_in // 128):
            if _is_act_tile(kt):
                cols = slice(kt * 128, (kt + 1) * 128)
                corr += wb[:, cols].sum(axis=1) * 0.5
                wb[:, cols] *= 0.5
        return wb, thr_next - corr

    w2b, thr2 = scaled(np.sign(w2).astype(f64), _thr(b2, g2, be2, m2, v2), 2048)
    w3b, thr3 = scaled(np.sign(w3).astype(f64), _thr(b3, g3, be3, m3, v3), 1024)
    w4b, thr4 = scaled(np.sign(w4).astype(f64), _thr(b4, g4, be4, m4, v4), 512)

    out = dict(
        w1t=np.ascontiguousarray(np.concatenate([w1b.T, w1b.T], axis=0).astype(bf)),  # [128,2048]
        w2t=_feat_major(w2b.T.astype(f8), 2048),                        # [128,16,1024]
        w3t=_feat_major(w3b.T.astype(f8), 1024),                        # [128,8,512]
        w4t=_feat_major(w4b.T.astype(f8), 512),                         # [128,4,64]
        w5t=np.ascontiguousarray(np.asarray(w5, np.float32).reshape(64, 1).astype(bf)),  # [64,1]
        thr1=_feat_major(thr1.astype(np.float32), 2048),
        thr2=_feat_major(thr2.astype(np.float32), 1024),
        thr3=_feat_major(thr3.astype(np.float32), 512),
        thr4=np.ascontiguousarray(thr4.astype(np.float32).reshape(64, 1)),
        nthr1=_feat_major((-thr1).astype(np.float32), 2048),
        nthr2=_feat_major((-thr2).astype(np.float32), 1024),
        nthr3=_feat_major((-thr3).astype(np.float32), 512),
        b5=np.asarray(b5, np.float32).reshape(1, 1),
    )
    return out


_CACHED = {}


def run(inputs, trace=False):
    if "nc" not in _CACHED:
        _CACHED["nc"] = build_program()
    nc = _CACHED["nc"]

    x = np.asarray(inputs["x"], np.float32)
    wmap = prep_weights(**{k: np.asarray(v) for k, v in inputs.items() if k != "x"})
    in_maps = []
    for c in range(N_CORES):
        m = dict(wmap)
        m["x"] = prep_x(x[c * BC : (c + 1) * BC])
        in_maps.append(m)

    res = run_bass_kernel_spmd(nc, in_maps, list(range(N_CORES)), trace=trace)
    out = np.concatenate(
        [np.asarray(r["out"]).reshape(BC, 1) for r in res.results], axis=0
    )
    return out, res


def kernel(**inputs) -> np.ndarray:
    out, _ = run(inputs, trace=False)
    return out


# revision 20
# speedup vs baseline: 1.2790x; 1.0493x over previous
"""Binarized MLP (64->2048->1024->512->64->1, B=32768) on 8 trn2 NeuronCores.

Strategy (data-parallel over batch, weights replicated):
- Activations after each binarized layer are exactly {0,1}; binarized weights
  are exactly {-1,0,+1}. Layers 2-4 run in fp8 DoubleRow with exact fp32 PSUM
  accumulation.
- x is transposed and 2-term bf16 split (residual ~|x|*2^-18) on the host,
  shipped as one [128, bc] bf16 tensor with both terms stacked on the
  partition dim, so each 128-feature output tile needs ONE K=128 bf16 matmul
  and the device does no transposes or split arithmetic at all. (fp8
  DoubleRow for L1 was tried and reverted: the DR adder tree accumulates at
  ~13-bit mantissa, exact for the integer-valued sums of L2-4 but too lossy
  for real-valued L1.) Layer 5 uses a single bf16 copy of w5 (error ~2^-9
  relative, washed out by the sigmoid under the rel-err budget).
- The next tile's L1 matmuls are interleaved between the current tile's L2
  m-tiles so L1's epilogues drain while the PE is busy with DoubleRow work
  instead of backpressuring it through the PSUM pool.
- BN(eval) + bias + hardtanh + 1-bit actq collapse into a per-feature
  threshold: out_bit = (matmul > thr), thr = m - be*sqrt(v+eps)/g - b.
- Activations are kept feature-major on chip ([feat, batch]).
- Threshold epilogues alternate DVE (is_gt -> {0,1}) and ACT (Sign -> {-1,+1})
  per 128-feature tile. ACT-coded features get next-layer weight columns
  scaled by 1/2 (exact in fp8) plus a host-side threshold correction.
- Per-tile sigmoid writes column slices of one [1, 4096] output tile so the
  rep loop issues a single output DMA.
"""

import sys

import numpy as np

sys.path.insert(0, "/opt/trn_rl_repo")

import ml_dtypes

import concourse.bacc as bacc
import concourse.mybir as mybir
import concourse.tile as tile
from concourse.bass import ts, ds
from concourse.bass_utils import run_bass_kernel_spmd
from contextlib import ExitStack

N_CORES = 8
B = 32768
BC = B // N_CORES          # 4096 rows per core
BT = 512                   # batch tile (free dim of matmuls)
EPS = 1e-5

F32 = mybir.dt.float32
BF16 = mybir.dt.bfloat16
FP8 = mybir.dt.float8e4
DR = mybir.MatmulPerfMode.DoubleRow


def _thr(b, g, be, m, v):
    # (z + b - m) * g/sqrt(v+eps) + be > 0  <=>  z > m - be*sqrt(v+eps)/g - b
    s = np.float64(g) / np.sqrt(np.float64(v) + EPS)
    return np.float64(m) - np.float64(be) / s - np.float64(b)


def _feat_major(a, n_feat):
    # [n_feat(, rest)] -> [128, n_feat//128(, rest)] with feature f at
    # [f % 128, f // 128]
    ks = n_feat // 128
    return np.ascontiguousarray(a.reshape((ks, 128) + a.shape[1:]).swapaxes(0, 1))


def prep_x(xc):
    """[bc, 64] f32 -> [128, bc] bf16: x^T 2-term bf16 split stacked on the
    partition dim (rows 0-63 high term, 64-127 residual term)."""
    bf = ml_dtypes.bfloat16
    xt = np.ascontiguousarray(xc.astype(np.float32).T)      # [64, bc]
    hi = xt.astype(bf)
    lo = (xt - hi.astype(np.float32)).astype(bf)
    return np.ascontiguousarray(np.concatenate([hi, lo], axis=0))


def _is_act_tile(kt):
    # m-tile kt of a layer's output features: DVE ({0,1}) if even, ACT ({-1,1})
    return kt % 2 == 1


def build_program(bc=BC, bt=BT, reps=1, dummies=True, interleave=True, psmm=8):
    nbt = bc // bt
    nc = bacc.Bacc("TRN2", target_bir_lowering=False)

    x_d = nc.declare_dram_parameter("x", [128, bc], BF16, False)
    w1t_d = nc.declare_dram_parameter("w1t", [128, 2048], BF16, False)
    w2t_d = nc.declare_dram_parameter("w2t", [128, 16, 1024], FP8, False)
    w3t_d = nc.declare_dram_parameter("w3t", [128, 8, 512], FP8, False)
    w4t_d = nc.declare_dram_parameter("w4t", [128, 4, 64], FP8, False)
    w5t_d = nc.declare_dram_parameter("w5t", [64, 1], BF16, False)
    thr1_d = nc.declare_dram_parameter("thr1", [128, 16], F32, False)
    thr2_d = nc.declare_dram_parameter("thr2", [128, 8], F32, False)
    thr3_d = nc.declare_dram_parameter("thr3", [128, 4], F32, False)
    thr4_d = nc.declare_dram_parameter("thr4", [64, 1], F32, False)
    nthr1_d = nc.declare_dram_parameter("nthr1", [128, 16], F32, False)
    nthr2_d = nc.declare_dram_parameter("nthr2", [128, 8], F32, False)
    nthr3_d = nc.declare_dram_parameter("nthr3", [128, 4], F32, False)
    b5_d = nc.declare_dram_parameter("b5", [1, 1], F32, False)
    out_d = nc.declare_dram_parameter("out", [nbt, bt], F32, True)

    gt = mybir.AluOpType.is_gt
    SIGN = mybir.ActivationFunctionType.Sign
    SIGMOID = mybir.ActivationFunctionType.Sigmoid

    with tile.TileContext(nc) as tc:
        with ExitStack() as ctx:
            const = ctx.enter_context(tc.tile_pool(name="const", bufs=1))
            xb_p = ctx.enter_context(tc.tile_pool(name="xb", bufs=3))
            h1_p = ctx.enter_context(tc.tile_pool(name="h1", bufs=3))
            h2_p = ctx.enter_context(tc.tile_pool(name="h2", bufs=3))
            h3_p = ctx.enter_context(tc.tile_pool(name="h3", bufs=2))
            h4_p = ctx.enter_context(tc.tile_pool(name="h4", bufs=2))
            o_p = ctx.enter_context(tc.tile_pool(name="o", bufs=2))
            ps_mm = ctx.enter_context(tc.tile_pool(name="psmm", bufs=psmm, space="PSUM"))

            def cload(nm, shape, dtype, dram):
                t = const.tile(shape, dtype, tag=nm, name=nm)
                nc.sync.dma_start(t[:], dram[:])
                return t

            w1t = cload("w1t", [128, 2048], BF16, w1t_d)
            w2t = cload("w2t", [128, 16, 1024], FP8, w2t_d)
            w3t = cload("w3t", [128, 8, 512], FP8, w3t_d)
            w4t = cload("w4t", [128, 4, 64], FP8, w4t_d)
            w5t = cload("w5t", [64, 1], BF16, w5t_d)
            thr1 = cload("thr1", [128, 16], F32, thr1_d)
            thr2 = cload("thr2", [128, 8], F32, thr2_d)
            thr3 = cload("thr3", [128, 4], F32, thr3_d)
            thr4 = cload("thr4", [64, 1], F32, thr4_d)
            nthr1 = cload("nthr1", [128, 16], F32, nthr1_d)
            nthr2 = cload("nthr2", [128, 8], F32, nthr2_d)
            nthr3 = cload("nthr3", [128, 4], F32, nthr3_d)
            b5 = cload("b5", [1, 1], F32, b5_d)

            # --- dummy consumers: absorb every const-producing semaphore so
            # steady-state matmuls/epilogues carry at most one wait each ---
            if dummies:
                dps = ps_mm.tile([128, bt], F32, tag="mm", name="dps")
                nc.tensor.matmul(dps[:, 0:128], lhsT=w1t[:, 0:128], rhs=w1t[:, 0:128],
                                 start=True, stop=True)
                nc.tensor.matmul(dps[:, 0:128], lhsT=w2t[:, 0, 0:128], rhs=w2t[:, 0, 0:128],
                                 start=True, stop=True)
                nc.tensor.matmul(dps[:, 0:128], lhsT=w3t[:, 0, 0:128], rhs=w3t[:, 0, 0:128],
                                 start=True, stop=True)
                nc.tensor.matmul(dps[:64, 0:64], lhsT=w4t[:, 0, :], rhs=w4t[:, 0, :],
                                 start=True, stop=True)
                nc.tensor.matmul(dps[:1, 0:1], lhsT=w5t[:], rhs=w5t[:],
                                 start=True, stop=True)
                dsb = const.tile([128, 16], F32)
                nc.vector.tensor_copy(dsb[:, 0:16], thr1[:])
                nc.vector.tensor_copy(dsb[:, 0:8], thr2[:])
                nc.vector.tensor_copy(dsb[:, 0:4], thr3[:])
                nc.vector.tensor_copy(dsb[:64, 0:1], thr4[:])
                dsb2 = const.tile([128, 16], F32)
                nc.scalar.copy(dsb2[:, 0:16], nthr1[:])
                nc.scalar.copy(dsb2[:, 0:8], nthr2[:])
                nc.scalar.copy(dsb2[:, 0:4], nthr3[:])
                nc.scalar.copy(dsb2[:1, 0:1], b5[:])

            def mm_layer(ps, w, h, mt, nk):
                msl = slice(None) if mt is None else ts(mt, 128)
                for k in range(0, nk, 2):
                    nc.tensor.matmul(ps[:], lhsT=w[:, k : k + 2, msl],
                                     rhs=h[:, k : k + 2, :], perf_mode=DR,
                                     start=(k == 0), stop=(k == nk - 2))

            def epilogue(h_ap, ps, mt, thr, nthr):
                if _is_act_tile(mt):
                    nc.scalar.activation(h_ap, ps[:], SIGN,
                                         bias=nthr[:, mt : mt + 1], scale=1.0)
                else:
                    nc.vector.tensor_scalar(h_ap, ps[:], thr[:, mt : mt + 1],
                                            None, gt)

            def front_a(b):
                """load the host-split x tile: rows 0-63 = bf16(x^T),
                rows 64-127 = bf16 residual (2-term split, done host-side)"""
                xb = xb_p.tile([128, bt], BF16, tag="xb", name="xb")
                nc.sync.dma_start(xb[:], x_d[:, ds(b * bt, bt)])
                return xb

            def l1_mts(h1, xb, mts):
                """L1 matmuls + thresholds for the given m-tiles -> h1"""
                for mt in mts:
                    ps = ps_mm.tile([128, bt], F32, tag="mm", name="ps")
                    nc.tensor.matmul(ps[:], lhsT=w1t[:, ts(mt, 128)], rhs=xb[:],
                                     start=True, stop=True)
                    epilogue(h1[:, mt, :], ps, mt, thr1, nthr1)

            def emit_l5(b, h4, o):
                """L5 matmul + sigmoid for tile b (deferred so the PE has L2
                work of the next tile while the L4 epilogue produces h4)."""
                ps5 = ps_mm.tile([1, bt], F32, tag="mm", name="ps5")
                nc.tensor.matmul(ps5[:], lhsT=w5t[:], rhs=h4[:],
                                 start=True, stop=True)
                nc.scalar.activation(o[:, ts(b, bt)], ps5[:], SIGMOID,
                                     bias=b5[:1, :1], scale=1.0)

            def stage_back(b, h1, h1_next, xb_next, o, prev_l5):
                """L2..L4 for batch tile b; returns (b, h4) for deferred L5.

                The next tile's L1 matmuls are interleaved between L2 m-tiles:
                during L2 the epilogue engines are underloaded, so the L1
                epilogues drain there instead of backpressuring the PE. The
                previous tile's L5+sigmoid is emitted after the first L2
                m-tile for the same reason."""
                h2 = h2_p.tile([128, 8, bt], FP8, tag="h2", name="h2")
                for mt in range(8):
                    ps = ps_mm.tile([128, bt], F32, tag="mm", name="ps")
                    mm_layer(ps, w2t, h1, mt, 16)
                    epilogue(h2[:, mt, :], ps, mt, thr2, nthr2)
                    if mt == 0 and prev_l5 is not None:
                        emit_l5(prev_l5[0], prev_l5[1], o)
                    if interleave and xb_next is not None:
                        l1_mts(h1_next, xb_next, [2 * mt, 2 * mt + 1])

                h3 = h3_p.tile([128, 4, bt], FP8, tag="h3", name="h3")
                for mt in range(4):
                    ps = ps_mm.tile([128, bt], F32, tag="mm", name="ps")
                    mm_layer(ps, w3t, h2, mt, 8)
                    epilogue(h3[:, mt, :], ps, mt, thr3, nthr3)

                h4 = h4_p.tile([64, bt], BF16, tag="h4", name="h4")
                ps4 = ps_mm.tile([64, bt], F32, tag="mm", name="ps4")
                mm_layer(ps4, w4t, h3, None, 4)
                nc.vector.tensor_scalar(h4[:], ps4[:], thr4[:, 0:1], None, gt)
                return (b, h4)

            rep_ctx = tc.For_i(0, reps, 1) if reps > 1 else None
            if rep_ctx is not None:
                rep_ctx.__enter__()

            o = o_p.tile([1, nbt * bt], F32, tag="o", name="o")
            h1_cur = h1_p.tile([128, 16, bt], FP8, tag="h1", name="h1")
            l1_mts(h1_cur, front_a(0), range(16))
            prev_l5 = None
            for b in range(nbt):
                if b + 1 < nbt:
                    xb_next = front_a(b + 1)
                    h1_next = h1_p.tile([128, 16, bt], FP8, tag="h1", name="h1")
                else:
                    xb_next = h1_next = None
                prev_l5 = stage_back(b, h1_cur, h1_next, xb_next, o, prev_l5)
                if not interleave and xb_next is not None:
                    l1_mts(h1_next, xb_next, range(16))
                h1_cur = h1_next
            emit_l5(prev_l5[0], prev_l5[1], o)
            nc.sync.dma_start(out_d[:], o[:])

            if rep_ctx is not None:
                rep_ctx.__exit__(None, None, None)

    nc.compile()
    return nc


def prep_weights(w1, b1, w2, b2, w3, b3, w4, b4, w5, b5,
                 *, g1, be1, m1, v1, g2, be2, m2, v2,
                 g3, be3, m3, v3, g4, be4, m4, v4):
    bf = ml_dtypes.bfloat16
    f8 = ml_dtypes.float8_e4m3
    f64 = np.float64

    # layer 1: sign weights duplicated on both K-halves (for the stacked
    # 2-term bf16 split of x); no input coding.
    w1b = np.sign(w1).astype(f64)                                       # [2048,64]
    thr1 = _thr(b1, g1, be1, m1, v1)                                    # [2048]

    def scaled(wb, thr_next, n_in):
        """Scale ACT-coded input columns by 1/2 and fold the matching
        -0.5*sum(sign) correction into the next layer's threshold."""
        wb = wb.copy()
        corr = np.zeros(wb.shape[0], f64)
        for kt in range(n_in // 128):
            if _is_act_tile(kt):
                cols = slice(kt * 128, (kt + 1) * 128)
                corr += wb[:, cols].sum(axis=1) * 0.5
                wb[:, cols] *= 0.5
        return wb, thr_next - corr

    w2b, thr2 = scaled(np.sign(w2).astype(f64), _thr(b2, g2, be2, m2, v2), 2048)
    w3b, thr3 = scaled(np.sign(w3).astype(f64), _thr(b3, g3, be3, m3, v3), 1024)
    w4b, thr4 = scaled(np.sign(w4).astype(f64), _thr(b4, g4, be4, m4, v4), 512)

    out = dict(
        w1t=np.ascontiguousarray(np.concatenate([w1b.T, w1b.T], axis=0).astype(bf)),  # [128,2048]
        w2t=_feat_major(w2b.T.astype(f8), 2048),                        # [128,16,1024]
        w3t=_feat_major(w3b.T.astype(f8), 1024),                        # [128,8,512]
        w4t=_feat_major(w4b.T.astype(f8), 512),                         # [128,4,64]
        w5t=np.ascontiguousarray(np.asarray(w5, np.float32).reshape(64, 1).astype(bf)),  # [64,1]
        thr1=_feat_major(thr1.astype(np.float32), 2048),
        thr2=_feat_major(thr2.astype(np.float32), 1024),
        thr3=_feat_major(thr3.astype(np.float32), 512),
        thr4=np.ascontiguousarray(thr4.astype(np.float32).reshape(64, 1)),
        nthr1=_feat_major((-thr1).astype(np.float32), 2048),
        nthr2=_feat_major((-thr2).astype(np.float32), 1024),
        nthr3=_feat_major((-thr3).astype(np.float32), 512),
        b5=np.asarray(b5, np.float32).reshape(1, 1),
    )
    return out


_CACHED = {}


def run(inputs, trace=False):
    if "nc" not in _CACHED:
        _CACHED["nc"] = build_program()
    nc = _CACHED["nc"]

    x = np.asarray(inputs["x"], np.float32)
    wmap = prep_weights(**{k: np.asarray(v) for k, v in inputs.items() if k != "x"})
    in_maps = []
    for c in range(N_CORES):
        m = dict(wmap)
        m["x"] = prep_x(x[c * BC : (c + 1) * BC])
        in_maps.append(m)

    res = run_bass_kernel_spmd(nc, in_maps, list(range(N_CORES)), trace=trace)
    out = np.concatenate(
        [np.asarray(r["out"]).reshape(BC, 1) for r in res.results], axis=0
    )
    return out, res


def kernel(**inputs) -> np.ndarray:
    out, _ = run(inputs, trace=False)
    return out


# revision 22
# speedup vs baseline: 1.3181x; 1.0306x over previous
"""Binarized MLP (64->2048->1024->512->64->1, B=32768) on 8 trn2 NeuronCores.

Strategy (data-parallel over batch, weights replicated):
- Activations after each binarized layer are exactly {0,1}; binarized weights
  are exactly {-1,0,+1}. Layers 2-4 run in fp8 DoubleRow with exact fp32 PSUM
  accumulation.
- x is transposed and 2-term bf16 split (residual ~|x|*2^-18) on the host,
  shipped as one [128, bc] bf16 tensor with both terms stacked on the
  partition dim, so each 128-feature output tile needs ONE K=128 bf16 matmul
  and the device does no transposes or split arithmetic at all. (fp8
  DoubleRow for L1 was tried and reverted: the DR adder tree accumulates at
  ~13-bit mantissa, exact for the integer-valued sums of L2-4 but too lossy
  for real-valued L1.) Layer 5 uses a single bf16 copy of w5 (error ~2^-9
  relative, washed out by the sigmoid under the rel-err budget).
- The next tile's L1 matmuls are interleaved between the current tile's L2
  m-tiles so L1's epilogues drain while the PE is busy with DoubleRow work
  instead of backpressuring it through the PSUM pool.
- BN(eval) + bias + hardtanh + 1-bit actq collapse into a per-feature
  threshold: out_bit = (matmul > thr), thr = m - be*sqrt(v+eps)/g - b.
- Activations are kept feature-major on chip ([feat, batch]).
- Threshold epilogues alternate DVE (is_gt -> {0,1}) and ACT (Sign -> {-1,+1})
  per 128-feature tile. ACT-coded features get next-layer weight columns
  scaled by 1/2 (exact in fp8) plus a host-side threshold correction.
- Per-tile sigmoid writes column slices of one [1, 4096] output tile so the
  rep loop issues a single output DMA.
"""

import sys

import numpy as np

sys.path.insert(0, "/opt/trn_rl_repo")

import ml_dtypes

import concourse.bacc as bacc
import concourse.mybir as mybir
import concourse.tile as tile
from concourse.bass import ts, ds
from concourse.bass_utils import run_bass_kernel_spmd
from contextlib import ExitStack

N_CORES = 8
B = 32768
BC = B // N_CORES          # 4096 rows per core
BT = 512                   # batch tile (free dim of matmuls)
EPS = 1e-5

F32 = mybir.dt.float32
BF16 = mybir.dt.bfloat16
FP8 = mybir.dt.float8e4
DR = mybir.MatmulPerfMode.DoubleRow


def _thr(b, g, be, m, v):
    # (z + b - m) * g/sqrt(v+eps) + be > 0  <=>  z > m - be*sqrt(v+eps)/g - b
    s = np.float64(g) / np.sqrt(np.float64(v) + EPS)
    return np.float64(m) - np.float64(be) / s - np.float64(b)


def _feat_major(a, n_feat):
    # [n_feat(, rest)] -> [128, n_feat//128(, rest)] with feature f at
    # [f % 128, f // 128]
    ks = n_feat // 128
    return np.ascontiguousarray(a.reshape((ks, 128) + a.shape[1:]).swapaxes(0, 1))


def prep_x(xc):
    """[bc, 64] f32 -> [128, bc] bf16: x^T 2-term bf16 split stacked on the
    partition dim (rows 0-63 high term, 64-127 residual term)."""
    bf = ml_dtypes.bfloat16
    xt = np.ascontiguousarray(xc.astype(np.float32).T)      # [64, bc]
    hi = xt.astype(bf)
    lo = (xt - hi.astype(np.float32)).astype(bf)
    return np.ascontiguousarray(np.concatenate([hi, lo], axis=0))


def _is_act_tile(kt):
    # m-tile kt of a layer's output features: DVE ({0,1}) if even, ACT ({-1,1})
    return kt % 2 == 1


def build_program(bc=BC, bt=BT, reps=1, dummies=True, interleave=True, psmm=8, hbufs=3):
    nbt = bc // bt
    nc = bacc.Bacc("TRN2", target_bir_lowering=False)

    x_d = nc.declare_dram_parameter("x", [128, bc], BF16, False)
    w1t_d = nc.declare_dram_parameter("w1t", [128, 2048], BF16, False)
    w2t_d = nc.declare_dram_parameter("w2t", [128, 16, 1024], FP8, False)
    w3t_d = nc.declare_dram_parameter("w3t", [128, 8, 512], FP8, False)
    w4t_d = nc.declare_dram_parameter("w4t", [128, 4, 64], FP8, False)
    w5t_d = nc.declare_dram_parameter("w5t", [64, 1], BF16, False)
    thr1_d = nc.declare_dram_parameter("thr1", [128, 16], F32, False)
    thr2_d = nc.declare_dram_parameter("thr2", [128, 8], F32, False)
    thr3_d = nc.declare_dram_parameter("thr3", [128, 4], F32, False)
    thr4_d = nc.declare_dram_parameter("thr4", [64, 1], F32, False)
    nthr1_d = nc.declare_dram_parameter("nthr1", [128, 16], F32, False)
    nthr2_d = nc.declare_dram_parameter("nthr2", [128, 8], F32, False)
    nthr3_d = nc.declare_dram_parameter("nthr3", [128, 4], F32, False)
    b5_d = nc.declare_dram_parameter("b5", [1, 1], F32, False)
    out_d = nc.declare_dram_parameter("out", [nbt, bt], F32, True)

    gt = mybir.AluOpType.is_gt
    SIGN = mybir.ActivationFunctionType.Sign
    SIGMOID = mybir.ActivationFunctionType.Sigmoid

    with tile.TileContext(nc) as tc:
        with ExitStack() as ctx:
            const = ctx.enter_context(tc.tile_pool(name="const", bufs=1))
            xb_p = ctx.enter_context(tc.tile_pool(name="xb", bufs=hbufs))
            h1_p = ctx.enter_context(tc.tile_pool(name="h1", bufs=hbufs))
            h2_p = ctx.enter_context(tc.tile_pool(name="h2", bufs=3))
            h3_p = ctx.enter_context(tc.tile_pool(name="h3", bufs=2))
            h4_p = ctx.enter_context(tc.tile_pool(name="h4", bufs=2))
            o_p = ctx.enter_context(tc.tile_pool(name="o", bufs=2))
            ps_mm = ctx.enter_context(tc.tile_pool(name="psmm", bufs=psmm, space="PSUM"))

            def cload(nm, shape, dtype, dram):
                t = const.tile(shape, dtype, tag=nm, name=nm)
                nc.sync.dma_start(t[:], dram[:])
                return t

            w1t = cload("w1t", [128, 2048], BF16, w1t_d)
            w2t = cload("w2t", [128, 16, 1024], FP8, w2t_d)
            w3t = cload("w3t", [128, 8, 512], FP8, w3t_d)
            w4t = cload("w4t", [128, 4, 64], FP8, w4t_d)
            w5t = cload("w5t", [64, 1], BF16, w5t_d)
            thr1 = cload("thr1", [128, 16], F32, thr1_d)
            thr2 = cload("thr2", [128, 8], F32, thr2_d)
            thr3 = cload("thr3", [128, 4], F32, thr3_d)
            thr4 = cload("thr4", [64, 1], F32, thr4_d)
            nthr1 = cload("nthr1", [128, 16], F32, nthr1_d)
            nthr2 = cload("nthr2", [128, 8], F32, nthr2_d)
            nthr3 = cload("nthr3", [128, 4], F32, nthr3_d)
            b5 = cload("b5", [1, 1], F32, b5_d)

            # --- dummy consumers: absorb every const-producing semaphore so
            # steady-state matmuls/epilogues carry at most one wait each ---
            if dummies:
                dps = ps_mm.tile([128, bt], F32, tag="mm", name="dps")
                nc.tensor.matmul(dps[:, 0:128], lhsT=w1t[:, 0:128], rhs=w1t[:, 0:128],
                                 start=True, stop=True)
                nc.tensor.matmul(dps[:, 0:128], lhsT=w2t[:, 0, 0:128], rhs=w2t[:, 0, 0:128],
                                 start=True, stop=True)
                nc.tensor.matmul(dps[:, 0:128], lhsT=w3t[:, 0, 0:128], rhs=w3t[:, 0, 0:128],
                                 start=True, stop=True)
                nc.tensor.matmul(dps[:64, 0:64], lhsT=w4t[:, 0, :], rhs=w4t[:, 0, :],
                                 start=True, stop=True)
                nc.tensor.matmul(dps[:1, 0:1], lhsT=w5t[:], rhs=w5t[:],
                                 start=True, stop=True)
                dsb = const.tile([128, 16], F32)
                nc.vector.tensor_copy(dsb[:, 0:16], thr1[:])
                nc.vector.tensor_copy(dsb[:, 0:8], thr2[:])
                nc.vector.tensor_copy(dsb[:, 0:4], thr3[:])
                nc.vector.tensor_copy(dsb[:64, 0:1], thr4[:])
                dsb2 = const.tile([128, 16], F32)
                nc.scalar.copy(dsb2[:, 0:16], nthr1[:])
                nc.scalar.copy(dsb2[:, 0:8], nthr2[:])
                nc.scalar.copy(dsb2[:, 0:4], nthr3[:])
                nc.scalar.copy(dsb2[:1, 0:1], b5[:])

            def mm_layer(ps, w, h, mt, nk):
                msl = slice(None) if mt is None else ts(mt, 128)
                for k in range(0, nk, 2):
                    nc.tensor.matmul(ps[:], lhsT=w[:, k : k + 2, msl],
                                     rhs=h[:, k : k + 2, :], perf_mode=DR,
                                     start=(k == 0), stop=(k == nk - 2))

            def epilogue(h_ap, ps, mt, thr, nthr):
                if _is_act_tile(mt):
                    nc.scalar.activation(h_ap, ps[:], SIGN,
                                         bias=nthr[:, mt : mt + 1], scale=1.0)
                else:
                    nc.vector.tensor_scalar(h_ap, ps[:], thr[:, mt : mt + 1],
                                            None, gt)

            def front_a(b):
                """load the host-split x tile: rows 0-63 = bf16(x^T),
                rows 64-127 = bf16 residual (2-term split, done host-side)"""
                xb = xb_p.tile([128, bt], BF16, tag="xb", name="xb")
                nc.sync.dma_start(xb[:], x_d[:, ds(b * bt, bt)])
                return xb

            def l1_mts(h1, xb, mts):
                """L1 matmuls + thresholds for the given m-tiles -> h1"""
                for mt in mts:
                    ps = ps_mm.tile([128, bt], F32, tag="mm", name="ps")
                    nc.tensor.matmul(ps[:], lhsT=w1t[:, ts(mt, 128)], rhs=xb[:],
                                     start=True, stop=True)
                    epilogue(h1[:, mt, :], ps, mt, thr1, nthr1)

            def emit_l5(b, h4, o):
                """L5 matmul + sigmoid for tile b (deferred so the PE has L2
                work of the next tile while the L4 epilogue produces h4)."""
                ps5 = ps_mm.tile([1, bt], F32, tag="mm", name="ps5")
                nc.tensor.matmul(ps5[:], lhsT=w5t[:], rhs=h4[:],
                                 start=True, stop=True)
                nc.scalar.activation(o[:, ts(b, bt)], ps5[:], SIGMOID,
                                     bias=b5[:1, :1], scale=1.0)

            def stage_back(b, h1, h1_next, xb_next, o, prev_l5):
                """L2..L4 for batch tile b; returns (b, h4) for deferred L5.

                The next tile's L1 matmuls are interleaved between L2 m-tiles:
                during L2 the epilogue engines are underloaded, so the L1
                epilogues drain there instead of backpressuring the PE. The
                previous tile's L5+sigmoid is emitted after the first L2
                m-tile for the same reason."""
                h2 = h2_p.tile([128, 8, bt], FP8, tag="h2", name="h2")
                for mt in range(8):
                    ps = ps_mm.tile([128, bt], F32, tag="mm", name="ps")
                    mm_layer(ps, w2t, h1, mt, 16)
                    epilogue(h2[:, mt, :], ps, mt, thr2, nthr2)
                    if mt == 0 and prev_l5 is not None:
                        emit_l5(prev_l5[0], prev_l5[1], o)
                    if interleave and xb_next is not None:
                        l1_mts(h1_next, xb_next, [2 * mt, 2 * mt + 1])

                h3 = h3_p.tile([128, 4, bt], FP8, tag="h3", name="h3")
                for mt in range(4):
                    ps = ps_mm.tile([128, bt], F32, tag="mm", name="ps")
                    mm_layer(ps, w3t, h2, mt, 8)
                    epilogue(h3[:, mt, :], ps, mt, thr3, nthr3)

                h4 = h4_p.tile([64, bt], BF16, tag="h4", name="h4")
                ps4 = ps_mm.tile([64, bt], F32, tag="mm", name="ps4")
                mm_layer(ps4, w4t, h3, None, 4)
                nc.vector.tensor_scalar(h4[:], ps4[:], thr4[:, 0:1], None, gt)
                return (b, h4)

            rep_ctx = tc.For_i(0, reps, 1) if reps > 1 else None
            if rep_ctx is not None:
                rep_ctx.__enter__()

            o = o_p.tile([1, nbt * bt], F32, tag="o", name="o")
            h1_cur = h1_p.tile([128, 16, bt], FP8, tag="h1", name="h1")
            xbs = [front_a(0), front_a(1)]      # prefetch distance 2
            l1_mts(h1_cur, xbs[0], range(16))
            prev_l5 = None
            for b in range(nbt):
                if b + 2 < nbt:
                    xbs.append(front_a(b + 2))
                if b + 1 < nbt:
                    xb_next = xbs[b + 1]
                    h1_next = h1_p.tile([128, 16, bt], FP8, tag="h1", name="h1")
                else:
                    xb_next = h1_next = None
                prev_l5 = stage_back(b, h1_cur, h1_next, xb_next, o, prev_l5)
                if not interleave and xb_next is not None:
                    l1_mts(h1_next, xb_next, range(16))
                h1_cur = h1_next
            emit_l5(prev_l5[0], prev_l5[1], o)
            nc.sync.dma_start(out_d[:], o[:])

            if rep_ctx is not None:
                rep_ctx.__exit__(None, None, None)

    nc.compile()
    return nc


def prep_weights(w1, b1, w2, b2, w3, b3, w4, b4, w5, b5,
                 *, g1, be1, m1, v1, g2, be2, m2, v2,
                 g3, be3, m3, v3, g4, be4, m4, v4):
    bf = ml_dtypes.bfloat16
    f8 = ml_dtypes.float8_e4m3
    f64 = np.float64

    # layer 1: sign weights duplicated on both K-halves (for the stacked
    # 2-term bf16 split of x); no input coding.
    w1b = np.sign(w1).astype(f64)                                       # [2048,64]
    thr1 = _thr(b1, g1, be1, m1, v1)                                    # [2048]

    def scaled(wb, thr_next, n_in):
        """Scale ACT-coded input columns by 1/2 and fold the matching
        -0.5*sum(sign) correction into the next layer's threshold."""
        wb = wb.copy()
        corr = np.zeros(wb.shape[0], f64)
        for kt in range(n_in // 128):
            if _is_act_tile(kt):
                cols = slice(kt * 128, (kt + 1) * 128)
                corr += wb[:, cols].sum(axis=1) * 0.5
                wb[:, cols] *= 0.5
        return wb, thr_next - corr

    w2b, thr2 = scaled(np.sign(w2).astype(f64), _thr(b2, g2, be2, m2, v2), 2048)
    w3b, thr3 = scaled(np.sign(w3).astype(f64), _thr(b3, g3, be3, m3, v3), 1024)
    w4b, thr4 = scaled(np.sign(w4).astype(f64), _thr(b4, g4, be4, m4, v4), 512)

    out = dict(
        w1t=np.ascontiguousarray(np.concatenate([w1b.T, w1b.T], axis=0).astype(bf)),  # [128,2048]
        w2t=_feat_major(w2b.T.astype(f8), 2048),                        # [128,16,1024]
        w3t=_feat_major(w3b.T.astype(f8), 1024),                        # [128,8,512]
        w4t=_feat_major(w4b.T.astype(f8), 512),                         # [128,4,64]
        w5t=np.ascontiguousarray(np.asarray(w5, np.float32).reshape(64, 1).astype(bf)),  # [64,1]
        thr1=_feat_major(thr1.astype(np.float32), 2048),
        thr2=_feat_major(thr2.astype(np.float32), 1024),
        thr3=_feat_major(thr3.astype(np.float32), 512),
        thr4=np.ascontiguousarray(thr4.astype(np.float32).reshape(64, 1)),
        nthr1=_feat_major((-thr1).astype(np.float32), 2048),
        nthr2=_feat_major((-thr2).astype(np.float32), 1024),
        nthr3=_feat_major((-thr3).astype(np.float32), 512),
        b5=np.asarray(b5, np.float32).reshape(1, 1),
    )
    return out


_CACHED = {}


def run(inputs, trace=False):
    if "nc" not in _CACHED:
        _CACHED["nc"] = build_program()
    nc = _CACHED["nc"]

    x = np.asarray(inputs["x"], np.float32)
    wmap = prep_weights(**{k: np.asarray(v) for k, v in inputs.items() if k != "x"})
    in_maps = []
    for c in range(N_CORES):
        m = dict(wmap)
        m["x"] = prep_x(x[c * BC : (c + 1) * BC])
        in_maps.append(m)

    res = run_bass_kernel_spmd(nc, in_maps, list(range(N_CORES)), trace=trace)
    out = np.concatenate(
        [np.asarray(r["out"]).reshape(BC, 1) for r in res.results], axis=0
    )
    return out, res


def kernel(**inputs) -> np.ndarray:
    out, _ = run(inputs, trace=False)
    return out


# revision 23
# speedup vs baseline: 1.3209x; 1.0021x over previous
"""Binarized MLP (64->2048->1024->512->64->1, B=32768) on 8 trn2 NeuronCores.

Strategy (data-parallel over batch, weights replicated):
- Activations after each binarized layer are exactly {0,1}; binarized weights
  are exactly {-1,0,+1}. Layers 2-4 run in fp8 DoubleRow with exact fp32 PSUM
  accumulation.
- x is transposed and 2-term bf16 split (residual ~|x|*2^-18) on the host,
  shipped as one [128, bc] bf16 tensor with both terms stacked on the
  partition dim, so each 128-feature output tile needs ONE K=128 bf16 matmul
  and the device does no transposes or split arithmetic at all. (fp8
  DoubleRow for L1 was tried and reverted: the DR adder tree accumulates at
  ~13-bit mantissa, exact for the integer-valued sums of L2-4 but too lossy
  for real-valued L1.) Layer 5 uses a single bf16 copy of w5 (error ~2^-9
  relative, washed out by the sigmoid under the rel-err budget).
- The next tile's L1 matmuls are interleaved between the current tile's L2
  m-tiles so L1's epilogues drain while the PE is busy with DoubleRow work
  instead of backpressuring it through the PSUM pool.
- BN(eval) + bias + hardtanh + 1-bit actq collapse into a per-feature
  threshold: out_bit = (matmul > thr), thr = m - be*sqrt(v+eps)/g - b.
- Activations are kept feature-major on chip ([feat, batch]).
- Threshold epilogues alternate DVE (is_gt -> {0,1}) and ACT (Sign -> {-1,+1})
  per 128-feature tile. ACT-coded features get next-layer weight columns
  scaled by 1/2 (exact in fp8) plus a host-side threshold correction.
- Per-tile sigmoid writes column slices of one [1, 4096] output tile so the
  rep loop issues a single output DMA.
"""

import sys

import numpy as np

sys.path.insert(0, "/opt/trn_rl_repo")

import ml_dtypes

import concourse.bacc as bacc
import concourse.mybir as mybir
import concourse.tile as tile
from concourse.bass import ts, ds
from concourse.bass_utils import run_bass_kernel_spmd
from contextlib import ExitStack

N_CORES = 8
B = 32768
BC = B // N_CORES          # 4096 rows per core
BT = 512                   # batch tile (free dim of matmuls)
EPS = 1e-5

F32 = mybir.dt.float32
BF16 = mybir.dt.bfloat16
FP8 = mybir.dt.float8e4
DR = mybir.MatmulPerfMode.DoubleRow


def _thr(b, g, be, m, v):
    # (z + b - m) * g/sqrt(v+eps) + be > 0  <=>  z > m - be*sqrt(v+eps)/g - b
    s = np.float64(g) / np.sqrt(np.float64(v) + EPS)
    return np.float64(m) - np.float64(be) / s - np.float64(b)


def _feat_major(a, n_feat):
    # [n_feat(, rest)] -> [128, n_feat//128(, rest)] with feature f at
    # [f % 128, f // 128]
    ks = n_feat // 128
    return np.ascontiguousarray(a.reshape((ks, 128) + a.shape[1:]).swapaxes(0, 1))


def prep_x(xc):
    """[bc, 64] f32 -> [128, bc] bf16: x^T 2-term bf16 split stacked on the
    partition dim (rows 0-63 high term, 64-127 residual term)."""
    bf = ml_dtypes.bfloat16
    xt = np.ascontiguousarray(xc.astype(np.float32).T)      # [64, bc]
    hi = xt.astype(bf)
    lo = (xt - hi.astype(np.float32)).astype(bf)
    return np.ascontiguousarray(np.concatenate([hi, lo], axis=0))


def _is_act_tile(kt):
    # m-tile kt of a layer's output features: DVE ({0,1}) if even, ACT ({-1,1})
    return kt % 2 == 1


def build_program(bc=BC, bt=BT, reps=1, dummies=True, interleave=True, psmm=8, hbufs=3):
    nbt = bc // bt
    nc = bacc.Bacc("TRN2", target_bir_lowering=False)

    x_d = nc.declare_dram_parameter("x", [128, bc], BF16, False)
    w1t_d = nc.declare_dram_parameter("w1t", [128, 2048], BF16, False)
    w2t_d = nc.declare_dram_parameter("w2t", [128, 16, 1024], FP8, False)
    w3t_d = nc.declare_dram_parameter("w3t", [128, 8, 512], FP8, False)
    w4t_d = nc.declare_dram_parameter("w4t", [128, 4, 64], FP8, False)
    w5t_d = nc.declare_dram_parameter("w5t", [64, 1], BF16, False)
    thr1_d = nc.declare_dram_parameter("thr1", [128, 16], F32, False)
    thr2_d = nc.declare_dram_parameter("thr2", [128, 8], F32, False)
    thr3_d = nc.declare_dram_parameter("thr3", [128, 4], F32, False)
    thr4_d = nc.declare_dram_parameter("thr4", [64, 1], F32, False)
    nthr1_d = nc.declare_dram_parameter("nthr1", [128, 16], F32, False)
    nthr2_d = nc.declare_dram_parameter("nthr2", [128, 8], F32, False)
    nthr3_d = nc.declare_dram_parameter("nthr3", [128, 4], F32, False)
    b5_d = nc.declare_dram_parameter("b5", [1, 1], F32, False)
    out_d = nc.declare_dram_parameter("out", [nbt, bt], F32, True)

    gt = mybir.AluOpType.is_gt
    SIGN = mybir.ActivationFunctionType.Sign
    SIGMOID = mybir.ActivationFunctionType.Sigmoid

    with tile.TileContext(nc) as tc:
        with ExitStack() as ctx:
            const = ctx.enter_context(tc.tile_pool(name="const", bufs=1))
            xb_p = ctx.enter_context(tc.tile_pool(name="xb", bufs=hbufs))
            h1_p = ctx.enter_context(tc.tile_pool(name="h1", bufs=hbufs))
            h2_p = ctx.enter_context(tc.tile_pool(name="h2", bufs=3))
            h3_p = ctx.enter_context(tc.tile_pool(name="h3", bufs=2))
            h4_p = ctx.enter_context(tc.tile_pool(name="h4", bufs=2))
            o_p = ctx.enter_context(tc.tile_pool(name="o", bufs=2))
            ps_mm = ctx.enter_context(tc.tile_pool(name="psmm", bufs=psmm, space="PSUM"))

            def cload(nm, shape, dtype, dram):
                t = const.tile(shape, dtype, tag=nm, name=nm)
                nc.sync.dma_start(t[:], dram[:])
                return t

            w1t = cload("w1t", [128, 2048], BF16, w1t_d)
            w2t = cload("w2t", [128, 16, 1024], FP8, w2t_d)
            w3t = cload("w3t", [128, 8, 512], FP8, w3t_d)
            w4t = cload("w4t", [128, 4, 64], FP8, w4t_d)
            w5t = cload("w5t", [64, 1], BF16, w5t_d)
            thr1 = cload("thr1", [128, 16], F32, thr1_d)
            thr2 = cload("thr2", [128, 8], F32, thr2_d)
            thr3 = cload("thr3", [128, 4], F32, thr3_d)
            thr4 = cload("thr4", [64, 1], F32, thr4_d)
            nthr1 = cload("nthr1", [128, 16], F32, nthr1_d)
            nthr2 = cload("nthr2", [128, 8], F32, nthr2_d)
            nthr3 = cload("nthr3", [128, 4], F32, nthr3_d)
            b5 = cload("b5", [1, 1], F32, b5_d)

            # --- dummy consumers: absorb every const-producing semaphore so
            # steady-state matmuls/epilogues carry at most one wait each ---
            if dummies:
                dps = ps_mm.tile([128, bt], F32, tag="mm", name="dps")
                nc.tensor.matmul(dps[:, 0:128], lhsT=w1t[:, 0:128], rhs=w1t[:, 0:128],
                                 start=True, stop=True)
                nc.tensor.matmul(dps[:, 0:128], lhsT=w2t[:, 0, 0:128], rhs=w2t[:, 0, 0:128],
                                 start=True, stop=True)
                nc.tensor.matmul(dps[:, 0:128], lhsT=w3t[:, 0, 0:128], rhs=w3t[:, 0, 0:128],
                                 start=True, stop=True)
                nc.tensor.matmul(dps[:64, 0:64], lhsT=w4t[:, 0, :], rhs=w4t[:, 0, :],
                                 start=True, stop=True)
                nc.tensor.matmul(dps[:1, 0:1], lhsT=w5t[:], rhs=w5t[:],
                                 start=True, stop=True)
                dsb = const.tile([128, 16], F32)
                nc.vector.tensor_copy(dsb[:, 0:16], thr1[:])
                nc.vector.tensor_copy(dsb[:, 0:8], thr2[:])
                nc.vector.tensor_copy(dsb[:, 0:4], thr3[:])
                nc.vector.tensor_copy(dsb[:64, 0:1], thr4[:])
                dsb2 = const.tile([128, 16], F32)
                nc.scalar.copy(dsb2[:, 0:16], nthr1[:])
                nc.scalar.copy(dsb2[:, 0:8], nthr2[:])
                nc.scalar.copy(dsb2[:, 0:4], nthr3[:])
                nc.scalar.copy(dsb2[:1, 0:1], b5[:])

            def mm_layer(ps, w, h, mt, nk):
                msl = slice(None) if mt is None else ts(mt, 128)
                for k in range(0, nk, 2):
                    nc.tensor.matmul(ps[:], lhsT=w[:, k : k + 2, msl],
                                     rhs=h[:, k : k + 2, :], perf_mode=DR,
                                     start=(k == 0), stop=(k == nk - 2))

            def epilogue(h_ap, ps, mt, thr, nthr):
                if _is_act_tile(mt):
                    nc.scalar.activation(h_ap, ps[:], SIGN,
                                         bias=nthr[:, mt : mt + 1], scale=1.0)
                else:
                    nc.vector.tensor_scalar(h_ap, ps[:], thr[:, mt : mt + 1],
                                            None, gt)

            def front_a(b):
                """load the host-split x tile: rows 0-63 = bf16(x^T),
                rows 64-127 = bf16 residual (2-term split, done host-side)"""
                xb = xb_p.tile([128, bt], BF16, tag="xb", name="xb")
                nc.sync.dma_start(xb[:], x_d[:, ds(b * bt, bt)])
                return xb

            def l1_mts(h1, xb, mts):
                """L1 matmuls + thresholds for the given m-tiles -> h1"""
                for mt in mts:
                    ps = ps_mm.tile([128, bt], F32, tag="mm", name="ps")
                    nc.tensor.matmul(ps[:], lhsT=w1t[:, ts(mt, 128)], rhs=xb[:],
                                     start=True, stop=True)
                    epilogue(h1[:, mt, :], ps, mt, thr1, nthr1)

            def emit_l5(b, h4, o):
                """L5 matmul + sigmoid for tile b (deferred so the PE has L2
                work of the next tile while the L4 epilogue produces h4)."""
                ps5 = ps_mm.tile([1, bt], F32, tag="mm", name="ps5")
                nc.tensor.matmul(ps5[:], lhsT=w5t[:], rhs=h4[:],
                                 start=True, stop=True)
                nc.scalar.activation(o[:, ts(b, bt)], ps5[:], SIGMOID,
                                     bias=b5[:1, :1], scale=1.0)

            def stage_back(b, h1, h1_next, xb_next, o, prev_l5):
                """L2..L4 for batch tile b; returns (b, h4) for deferred L5.

                The next tile's L1 matmuls are interleaved between L2 m-tiles:
                during L2 the epilogue engines are underloaded, so the L1
                epilogues drain there instead of backpressuring the PE. The
                previous tile's L5+sigmoid is emitted after the first L2
                m-tile for the same reason."""
                h2 = h2_p.tile([128, 8, bt], FP8, tag="h2", name="h2")
                for mt in range(8):
                    if interleave and xb_next is not None:
                        l1_mts(h1_next, xb_next, [2 * mt, 2 * mt + 1])
                    ps = ps_mm.tile([128, bt], F32, tag="mm", name="ps")
                    mm_layer(ps, w2t, h1, mt, 16)
                    epilogue(h2[:, mt, :], ps, mt, thr2, nthr2)
                    if mt == 0 and prev_l5 is not None:
                        emit_l5(prev_l5[0], prev_l5[1], o)

                h3 = h3_p.tile([128, 4, bt], FP8, tag="h3", name="h3")
                for mt in range(4):
                    ps = ps_mm.tile([128, bt], F32, tag="mm", name="ps")
                    mm_layer(ps, w3t, h2, mt, 8)
                    epilogue(h3[:, mt, :], ps, mt, thr3, nthr3)

                h4 = h4_p.tile([64, bt], BF16, tag="h4", name="h4")
                ps4 = ps_mm.tile([64, bt], F32, tag="mm", name="ps4")
                mm_layer(ps4, w4t, h3, None, 4)
                nc.vector.tensor_scalar(h4[:], ps4[:], thr4[:, 0:1], None, gt)
                return (b, h4)

            rep_ctx = tc.For_i(0, reps, 1) if reps > 1 else None
            if rep_ctx is not None:
                rep_ctx.__enter__()

            o = o_p.tile([1, nbt * bt], F32, tag="o", name="o")
            h1_cur = h1_p.tile([128, 16, bt], FP8, tag="h1", name="h1")
            xbs = [front_a(0), front_a(1)]      # prefetch distance 2
            l1_mts(h1_cur, xbs[0], range(16))
            prev_l5 = None
            for b in range(nbt):
                if b + 2 < nbt:
                    xbs.append(front_a(b + 2))
                if b + 1 < nbt:
                    xb_next = xbs[b + 1]
                    h1_next = h1_p.tile([128, 16, bt], FP8, tag="h1", name="h1")
                else:
                    xb_next = h1_next = None
                prev_l5 = stage_back(b, h1_cur, h1_next, xb_next, o, prev_l5)
                if not interleave and xb_next is not None:
                    l1_mts(h1_next, xb_next, range(16))
                h1_cur = h1_next
            emit_l5(prev_l5[0], prev_l5[1], o)
            nc.sync.dma_start(out_d[:], o[:])

            if rep_ctx is not None:
                rep_ctx.__exit__(None, None, None)

    nc.compile()
    return nc


def prep_weights(w1, b1, w2, b2, w3, b3, w4, b4, w5, b5,
                 *, g1, be1, m1, v1, g2, be2, m2, v2,
                 g3, be3, m3, v3, g4, be4, m4, v4):
    bf = ml_dtypes.bfloat16
    f8 = ml_dtypes.float8_e4m3
    f64 = np.float64

    # layer 1: sign weights duplicated on both K-halves (for the stacked
    # 2-term bf16 split of x); no input coding.
    w1b = np.sign(w1).astype(f64)                                       # [2048,64]
    thr1 = _thr(b1, g1, be1, m1, v1)                                    # [2048]

    def scaled(wb, thr_next, n_in):
        """Scale ACT-coded input columns by 1/2 and fold the matching
        -0.5*sum(sign) correction into the next layer's threshold."""
        wb = wb.copy()
        corr = np.zeros(wb.shape[0], f64)
        for kt in range(n_in // 128):
            if _is_act_tile(kt):
                cols = slice(kt * 128, (kt + 1) * 128)
                corr += wb[:, cols].sum(axis=1) * 0.5
                wb[:, cols] *= 0.5
        return wb, thr_next - corr

    w2b, thr2 = scaled(np.sign(w2).astype(f64), _thr(b2, g2, be2, m2, v2), 2048)
    w3b, thr3 = scaled(np.sign(w3).astype(f64), _thr(b3, g3, be3, m3, v3), 1024)
    w4b, thr4 = scaled(np.sign(w4).astype(f64), _thr(b4, g4, be4, m4, v4), 512)

    out = dict(
        w1t=np.ascontiguousarray(np.concatenate([w1b.T, w1b.T], axis=0).astype(bf)),  # [128,2048]
        w2t=_feat_major(w2b.T.astype(f8), 2048),                        # [128,16,1024]
        w3t=_feat_major(w3b.T.astype(f8), 1024),                        # [128,8,512]
        w4t=_feat_major(w4b.T.astype(f8), 512),                         # [128,4,64]
        w5t=np.ascontiguousarray(np.asarray(w5, np.float32).reshape(64, 1).astype(bf)),  # [64,1]
        thr1=_feat_major(thr1.astype(np.float32), 2048),
        thr2=_feat_major(thr2.astype(np.float32), 1024),
        thr3=_feat_major(thr3.astype(np.float32), 512),
        thr4=np.ascontiguousarray(thr4.astype(np.float32).reshape(64, 1)),
        nthr1=_feat_major((-thr1).astype(np.float32), 2048),
        nthr2=_feat_major((-thr2).astype(np.float32), 1024),
        nthr3=_feat_major((-thr3).astype(np.float32), 512),
        b5=np.asarray(b5, np.float32).reshape(1, 1),
    )
    return out


_CACHED = {}


def run(inputs, trace=False):
    if "nc" not in _CACHED:
        _CACHED["nc"] = build_program()
    nc = _CACHED["nc"]

    x = np.asarray(inputs["x"], np.float32)
    wmap = prep_weights(**{k: np.asarray(v) for k, v in inputs.items() if k != "x"})
    in_maps = []
    for c in range(N_CORES):
        m = dict(wmap)
        m["x"] = prep_x(x[c * BC : (c + 1) * BC])
        in_maps.append(m)

    res = run_bass_kernel_spmd(nc, in_maps, list(range(N_CORES)), trace=trace)
    out = np.concatenate(
        [np.asarray(r["out"]).reshape(BC, 1) for r in res.results], axis=0
    )
    return out, res


def kernel(**inputs) -> np.ndarray:
    out, _ = run(inputs, trace=False)
    return out
